# revision 6
# baseline (speedup 1.0000x reference)
"""CorrelationLayer1D Trainium2 kernel.

out[b,d,h,w] = sum_c x1[b,c,h,w] * x2[b,c,h,w-80+d]  (zero where index < 0)
B=8, C=128, H=160, W=320, D=81 (MAX_DISP=40, pad=80).

Sharding: data-parallel over batch, one batch element per NeuronCore (8 cores).

Per-core algorithm (all on device):
  For each h and each 64-wide w-chunk k, the TensorEngine computes the local
  Gram rectangle  q[m',n'] = sum_c x1[c,64k+m'] * x2pad[c,64k+n']  where
  x2pad is x2 left-padded with 80 zero columns.  The output band is the
  diagonals  out[d, 64k+m'] = q[m', m'+d], d in [0,81).
  Diagonal extraction cannot be done by compute engines (no per-partition
  offsets) nor by SBUF-side DMA APs (DGE resets the flat-step remainder at
  descriptor-run boundaries), but DRAM-side DMA APs with arbitrary outer
  strides and contiguous inner runs DO work.  So: bounce q to DRAM, re-load
  with a diagonal AP giving S[w-part, d-free], transpose on the PE via an
  identity matmul to T[d-part, w-free], and store with w contiguous.
"""

import numpy as np

B, C, H, W = 8, 128, 160, 320
D = 81
PAD = 80
MK = 64          # w-chunk width (matmul M)
NK = MK + PAD    # gram rectangle width (144)
NCHUNK = W // MK # 5
NH = 8           # h-group size
NGROUP = H // NH # 20
WP = W + PAD     # padded x2 row width (400)

_CACHE = {}


def _build_nc():
    import concourse.bass as bass
    import concourse.bacc as bacc
    import concourse.mybir as mybir
    from concourse import tile

    f32 = mybir.dt.float32
    nc = bacc.Bacc()

    x1 = nc.dram_tensor("x1", [C, H, W], f32, kind="ExternalInput")
    x2 = nc.dram_tensor("x2", [C, H, W], f32, kind="ExternalInput")
    ident = nc.dram_tensor("ident", [128, 128], f32, kind="ExternalInput")
    out = nc.dram_tensor("out", [D, H, W], f32, kind="ExternalOutput")

    with tile.TileContext(nc) as tc:
        with (
            tc.tile_pool(name="inpool", bufs=2) as inpool,
            tc.tile_pool(name="qpool", bufs=3) as qpool,
            tc.tile_pool(name="spool", bufs=3) as spool,
            tc.tile_pool(name="tpool", bufs=2) as tpool,
            tc.tile_pool(name="const", bufs=1) as constpool,
            tc.tile_pool(name="psq", bufs=4, space=bass.MemorySpace.PSUM) as psq,
            tc.tile_pool(name="pst", bufs=4, space=bass.MemorySpace.PSUM) as pst,
            tc.tile_pool(name="qdram", bufs=4, space="DRAM") as qdram,
        ):
            id_t = constpool.tile([128, 128], f32)
            nc.sync.dma_start(id_t[:, :], ident[:, :])

            for g in range(NGROUP):
                h0 = g * NH
                # ---- load inputs for this h-group ----
                x1_t = inpool.tile([C, NH, W], f32, tag="x1t")
                nc.sync.dma_start(x1_t[:, :, :], x1[:, h0 : h0 + NH, :])
                # x2 goes into a padded layout: [C, NH, WP], first PAD cols zero
                x2_t = inpool.tile([C, NH, WP], f32, tag="x2t")
                nc.vector.memset(x2_t[:, :, 0:PAD], 0.0)
                nc.sync.dma_start(x2_t[:, :, PAD:WP], x2[:, h0 : h0 + NH, :])

                t_t = tpool.tile([D, NH, W], f32, tag="t")
                for k in range(NCHUNK):
                    # ---- gram rectangles for all h in group ----
                    q_t = qpool.tile([MK, NH, NK], f32, tag="q")
                    for hh in range(NH):
                        q_ps = psq.tile([MK, NK], f32, tag="qps")
                        nc.tensor.matmul(
                            q_ps[:, :],
                            x1_t[:, hh, k * MK : k * MK + MK],
                            x2_t[:, hh, k * MK : k * MK + NK],
                        )
                        nc.vector.tensor_copy(q_t[:, hh, :], q_ps[:, :])
                    # ---- bounce to DRAM ----
                    q_d = qdram.tile([MK, NH, NK], f32, tag="qd")
                    nc.sync.dma_start(q_d[:, :, :], q_t[:, :, :])
                    # ---- diagonal re-load: S[m', hh, e] = q_d[m', hh, m'+e] ----
                    s_t = spool.tile([MK, NH, D], f32, tag="s")
                    diag_src = bass.AP(
                        q_d.tensor,
                        q_d.offset,
                        [[NH * NK + 1, MK], [NK, NH], [1, D]],
                    )
                    nc.sync.dma_start(s_t[:, :, :], diag_src)
                    # ---- transpose S -> T via identity matmul, stash in sb ----
                    for hh in range(NH):
                        t_ps = pst.tile([D, MK], f32, tag="tps")
                        nc.tensor.matmul(
                            t_ps[:, :],
                            s_t[:, hh, :],
                            id_t[0:MK, 0:MK],
                        )
                        nc.vector.tensor_copy(
                            t_t[:, hh, k * MK : k * MK + MK], t_ps[:, :]
                        )
                # ---- store the whole h-group ----
                nc.sync.dma_start(out[:, h0 : h0 + NH, :], t_t[:, :, :])

    nc.compile()
    return nc


def _get_nc():
    if "nc" not in _CACHE:
        _CACHE["nc"] = _build_nc()
    return _CACHE["nc"]


def kernel(x_1: np.ndarray, x_2: np.ndarray) -> np.ndarray:
    from concourse.bass_utils import run_bass_kernel_spmd

    nc = _get_nc()
    x_1 = np.ascontiguousarray(x_1, dtype=np.float32)
    x_2 = np.ascontiguousarray(x_2, dtype=np.float32)
    ident = np.eye(128, dtype=np.float32)
    in_maps = [
        {"x1": x_1[b], "x2": x_2[b], "ident": ident} for b in range(B)
    ]
    res = run_bass_kernel_spmd(nc, in_maps, list(range(B)))
    return np.stack([res.results[b]["out"] for b in range(B)], axis=0)
